# revision 31
# baseline (speedup 1.0000x reference)
"""DepGCN Trainium2 kernel.

Math (derived from the reference):
  The attention scores p[b,l,j] = text_score[b,l] + s_table[labels[b,l,j]] + sum(b_attn)
  are softmaxed over j.  Row-constant terms cancel in softmax, so with
  E[c] = exp(s_table[c] - max(s_table)), the softmax weights are
      w[l,j] = mask[l,j] * E[labels[l,j]] / rowsum[l],
      rowsum[l] = sum_j mask[l,j] * E[labels[l,j]].
  The aggregation sum_j w[l,j] * dep_emb[labels[l,j],:] @ W_fc + b_fc collapses
  onto the class histogram n[l,c] = #{j : mask[l,j] and labels[l,j]==c}:
      out = relu(text + (n @ G2) / rowsum),   rowsum = n @ E,
      G2[c,:] = E[c] * (dep_emb[c,:] @ W_fc + b_fc).
  Everything except the histogram is tiny.  The kernel computes the masked
  histogram on-device, one sample per NeuronCore (8 cores, B=8).
"""

import os
from contextlib import ExitStack

import numpy as np

import concourse.bass as bass
import concourse.tile as tile
from concourse import mybir
from concourse.bass_utils import run_bass_kernel_spmd

f32 = mybir.dt.float32
i32 = mybir.dt.int32
i8 = mybir.dt.int8

L = 256          # tokens per sample (rows and neighbor dim)
NF = 256         # feature dim
NCLS = 50        # dep label classes
KPAD = 64        # padded class (contraction) dim
B = 8            # batch = number of cores

AX = mybir.AxisListType
OP = mybir.AluOpType

HIST_MODE = os.environ.get("HIST_MODE", "v4")
# PROBE: "" (full body), "empty" (1 memset), "dma" (DMAs only),
# "hist" (DMAs + prep + histogram, no post), "nodma" (compute only)
PROBE = os.environ.get("PROBE", "")
STAGGERED = os.environ.get("STAGGERED", "0") == "1"

K80 = 0x80808080 - (1 << 32)     # int32 bit pattern of 0x80808080
NPAGE = NCLS + 1                  # dummy no-match page 0 + 50 class pages
W3 = 87                           # words per partition, 3 labels/word
PK3 = W3 * 3                      # packed label slots (261 >= 256)


_HIST3_OP = None


def _register_hist3():
    """Custom DVE op: per-class masked-label counts via SWAR byte compare +
    running prefix sum.  Src0 = packed label words (stride-0 repeated per
    class page), Src1 = per-page class quad (broadcast along words),
    C0 = 0x80808080.  z = (k80 - (Src0 ^ Src1)) & k80 marks matching byte
    lanes at bits 7/15/23; out = prefix sum of z.  Page-end differences give
    per-class lane counts (exact: byte lanes <= 0x7f so the subtract never
    borrows across lanes; lane totals < 2^7)."""
    global _HIST3_OP
    if _HIST3_OP is not None:
        return _HIST3_OP
    from concourse.dve_spec import Spec, Src0, Src1, C0, AluOp, Bin, scan
    from concourse import dve_ops
    from concourse.dve_ops import DveOp, OPS, _SUB_OPCODE_FOR_NAME
    from concourse.dve_uop import DveOpSpec
    from concourse.dve_spec import lower

    t = Bin(AluOp.BITWISE_XOR, Src0, Src1)
    d = Bin(AluOp.SUBTRACT, C0, t)
    z = Bin(AluOp.BITWISE_AND, d, C0)
    body = scan(AluOp.ADD, z)

    def _ref(in0, in1=None, s0=None, s1=None, imm2=None):
        i0 = np.asarray(in0).view(np.int32).astype(np.int64) & 0xFFFFFFFF
        i1 = np.asarray(in1).view(np.int32).astype(np.int64) & 0xFFFFFFFF
        k80 = 0x80808080
        z = ((k80 - (i0 ^ i1)) & 0xFFFFFFFF) & k80
        out = np.cumsum(z.reshape(z.shape[0], -1), axis=1) & 0xFFFFFFFF
        return out.reshape(z.shape).astype(np.uint32).view(np.int32)

    spec = Spec(body=body, reference=_ref)
    name = "HIST3_SWAR_ANT"
    row = max(_SUB_OPCODE_FOR_NAME.values()) + 1
    sha = {}
    for ver in ("v3", "v4"):
        try:
            sha[ver] = DveOpSpec(
                name=name, opcode=row, uops=lower(spec, ver=ver), rd1_en=True
            ).sha(ver)
        except Exception:
            pass
    op = DveOp(name, spec, subdim=False, uops_sha=sha)
    OPS.append(op)
    _SUB_OPCODE_FOR_NAME[name] = row
    _HIST3_OP = op
    return op


def _build_nc_v2(reps=1):
    """v2: host-packed bf16 inputs, single fused label/mask DMA, PE transpose,
    ACT for PSUM->SBUF + scaled copy + relu, optional GPSIMD histogram share."""
    bf16 = mybir.dt.bfloat16
    AF = mybir.ActivationFunctionType
    ngp = int(os.environ.get("HIST_GP", "0"))
    unroll = int(os.environ.get("UNROLL", "2"))
    nc = bass.Bass()
    lme = nc.dram_tensor("lm", [128, 4 * L], bf16, kind="ExternalInput")
    txte = nc.dram_tensor("txt", [128, 2 * NF], bf16, kind="ExternalInput")
    gext = nc.dram_tensor("gext", [KPAD, NF + 1], f32, kind="ExternalInput")
    ident = nc.dram_tensor("ident", [128, 128], f32, kind="ExternalInput")
    actc = nc.dram_tensor("actc", [128, KPAD], f32, kind="ExternalInput")
    # partition-major layout; host reassembles rows (free on host)
    out = nc.dram_tensor("out", [128, 2 * NF], f32, kind="ExternalOutput")
    out_r = out[:]

    with ExitStack() as ctx:
        tc = ctx.enter_context(tile.TileContext(nc))
        const = ctx.enter_context(tc.tile_pool(name="const", bufs=1))
        work = ctx.enter_context(tc.tile_pool(name="work", bufs=3))
        psum = ctx.enter_context(tc.tile_pool(name="psum", bufs=2, space="PSUM"))

        g_dma = const.tile([KPAD, NF + 1], f32)
        nc.sync.dma_start(g_dma[:], gext[:])
        g_sb = const.tile([KPAD, NF + 1], f32)
        nc.vector.tensor_copy(g_sb[:], g_dma[:])
        id_dma = const.tile([128, 128], f32)
        nc.sync.dma_start(id_dma[:], ident[:])
        id_sb = const.tile([128, 128], f32)
        nc.vector.tensor_copy(id_sb[:], id_dma[:])
        ac_dma = const.tile([128, KPAD], f32)
        nc.sync.dma_start(ac_dma[:], actc[:])
        ac_sb = const.tile([128, KPAD], f32)
        nc.vector.tensor_copy(ac_sb[:], ac_dma[:])

        def body():
            lm = work.tile([128, 4 * L], bf16, tag="lm")
            nc.sync.dma_start(lm[:], lme[:])
            txt = work.tile([128, 2 * NF], bf16, tag="txt")
            nc.sync.dma_start(txt[:], txte[:])

            # xm = (lab + 1) * mask  (masked -> 0, else label+1 in 1..50)
            xm = work.tile([128, 2 * L], bf16, tag="xm")
            nc.vector.scalar_tensor_tensor(
                xm[:], lm[:, 0 : 2 * L], 1.0, lm[:, 2 * L : 4 * L],
                op0=OP.add, op1=OP.mult,
            )

            # histogram n[l, 64*t + c] = #{j in tile t: xm == c+1}
            # classes split across engines: DVE is_equal+accum, GPSIMD
            # is_equal+accum, ACT Square->Exp(+accum) (exp(-16(x-v)^2) is 1
            # at x==v and <=1.1e-7 otherwise; sum error <=3e-5 per count).
            nact = int(os.environ.get("HIST_ACT", "0"))
            n = work.tile([128, 128], f32, tag="n")
            nc.vector.memset(n[:], 0.0)
            oh = work.tile([128, L], bf16, tag="oh")
            ndve = NCLS - ngp - nact
            if ngp:
                oh_g = work.tile([128, L], bf16, tag="oh_g")
                n_g = work.tile([128, 2 * ngp], f32, tag="n_g")
            if nact:
                q_a = work.tile([128, L], bf16, tag="q_a")
                e_a = work.tile([128, L], bf16, tag="e_a")
                n_a = work.tile([128, 2 * nact], f32, tag="n_a")
            for t in range(2):
                sl = slice(t * L, (t + 1) * L)
                for c in range(ndve):
                    nc.vector.tensor_scalar(
                        oh[:], xm[:, sl], float(c + 1), 0.0,
                        op0=OP.is_equal, op1=OP.add,
                        accum_out=n[:, 64 * t + c : 64 * t + c + 1],
                    )
                for i, c in enumerate(range(ndve, ndve + ngp)):
                    nc.gpsimd.tensor_scalar(
                        oh_g[:], xm[:, sl], float(c + 1), 0.0,
                        op0=OP.is_equal, op1=OP.add,
                        accum_out=n_g[:, t * ngp + i : t * ngp + i + 1],
                    )
                for i, c in enumerate(range(ndve + ngp, NCLS)):
                    nc.scalar.activation(
                        q_a[:], xm[:, sl], AF.Square, bias=ac_sb[:, c : c + 1]
                    )
                    nc.scalar.activation(
                        e_a[:], q_a[:], AF.Exp, scale=-16.0,
                        accum_out=n_a[:, t * nact + i : t * nact + i + 1],
                    )
            if ngp:
                for t in range(2):
                    nc.vector.tensor_copy(
                        n[:, 64 * t + ndve : 64 * t + ndve + ngp],
                        n_g[:, t * ngp : (t + 1) * ngp],
                    )
            if nact:
                for t in range(2):
                    nc.vector.tensor_copy(
                        n[:, 64 * t + ndve + ngp : 64 * t + NCLS],
                        n_a[:, t * nact : (t + 1) * nact],
                    )

            # per-tile transposed histograms side by side: ntp[c, 128*t + l]
            ntp = psum.tile([KPAD, 256], f32, tag="ntp")
            for t in range(2):
                nc.tensor.transpose(
                    ntp[:, 128 * t : 128 * t + 128],
                    n[:, 64 * t : 64 * t + KPAD], id_sb[:],
                )
            nT = work.tile([KPAD, 256], f32, tag="nT")
            nc.scalar.copy(nT[:], ntp[:])

            t1b = work.tile([128, 2 * NF], bf16, tag="t1b")
            for t in range(2):
                y = psum.tile([128, NF + 1], f32, tag=f"y{t}")
                nc.tensor.matmul(
                    y[:], nT[:, 128 * t : 128 * t + 128], g_sb[:],
                    start=True, stop=True,
                )
                r = work.tile([128, 1], f32, tag=f"r{t}")
                nc.vector.reciprocal(r[:], y[:, NF : NF + 1])
                nc.scalar.activation(
                    t1b[:, t * NF : (t + 1) * NF], y[:, 0:NF], AF.Copy, scale=r[:]
                )
            t2 = work.tile([128, 2 * NF], bf16, tag="t2")
            nc.vector.tensor_tensor(t2[:], t1b[:], txt[:], op=OP.add)
            o = work.tile([128, 2 * NF], f32, tag="o")
            nc.scalar.activation(o[:], t2[:], AF.Relu)
            nc.sync.dma_start(out_r, o[:])

        if reps == 1:
            body()
        elif reps % unroll == 0 and unroll > 1:
            with tc.For_i(0, reps // unroll, 1, staggered_reset=STAGGERED):
                for _ in range(unroll):
                    body()
        else:
            with tc.For_i(0, reps, 1, staggered_reset=STAGGERED):
                body()

    return nc


def _build_nc_v4(reps=1):
    """v4: transposed layout (neighbor j on partitions), 50 full-rate DVE
    is_equal ops (no accum), per-class reductions as accumulating PE matmuls
    with a sliding one-hot stationary operand.  nT lands directly in PSUM
    [class, (jhalf, l)]; the final y-matmuls fold the two j-halves."""
    bf16 = mybir.dt.bfloat16
    AF = mybir.ActivationFunctionType
    unroll = int(os.environ.get("UNROLL", "4"))
    nc = bass.Bass()
    # lm: [p, 0:256]=labels[l, p] (j-half 0), [p, 256:512]=labels[l, 128+p],
    #     [p, 512:1024]= same for mask   (transposed: partition = neighbor j)
    lme = nc.dram_tensor("lm", [128, 4 * L], bf16, kind="ExternalInput")
    txte = nc.dram_tensor("txt", [128, 2 * NF], bf16, kind="ExternalInput")
    gext = nc.dram_tensor("gext", [KPAD, NF + 1], f32, kind="ExternalInput")
    ohce = nc.dram_tensor("ohc", [128, 128], bf16, kind="ExternalInput")
    out = nc.dram_tensor("out", [128, 2 * NF], f32, kind="ExternalOutput")

    with ExitStack() as ctx:
        tc = ctx.enter_context(tile.TileContext(nc))
        const = ctx.enter_context(tc.tile_pool(name="const", bufs=1))
        work = ctx.enter_context(tc.tile_pool(name="work", bufs=4))
        ohp = ctx.enter_context(tc.tile_pool(name="ohp", bufs=10))
        psum = ctx.enter_context(tc.tile_pool(name="psum", bufs=2, space="PSUM"))

        g_dma = const.tile([KPAD, NF + 1], f32)
        nc.sync.dma_start(g_dma[:], gext[:])
        g_sb = const.tile([KPAD, NF + 1], f32)
        nc.vector.tensor_copy(g_sb[:], g_dma[:])
        oc_dma = const.tile([128, 128], bf16)
        nc.sync.dma_start(oc_dma[:], ohce[:])
        oc_sb = const.tile([128, 128], bf16)
        nc.vector.tensor_copy(oc_sb[:], oc_dma[:])

        def body():
            lm = work.tile([128, 4 * L], bf16, tag="lm")
            nc.sync.dma_start(lm[:], lme[:])
            txt = work.tile([128, 2 * NF], bf16, tag="txt")
            nc.sync.dma_start(txt[:], txte[:])

            # xmT[j mod 128, (jhalf, l)] = (lab + 1) * mask, transposed
            xmT = work.tile([128, 2 * L], bf16, tag="xmT")
            nc.vector.scalar_tensor_tensor(
                xmT[:], lm[:, 0 : 2 * L], 1.0, lm[:, 2 * L : 4 * L],
                op0=OP.add, op1=OP.mult,
            )

            # nT[c, (jhalf, l)] accumulated on PE: for each class, one-hot
            # compare (DVE, 4x) then += e_c outer-product reduce (PE).
            # oc_sb col 63 is all-ones, so oc_sb[:, 63-c : 127-c] is the
            # [128, 64] stationary with a 1-column at position c.
            ntp = psum.tile([KPAD, 2 * L], f32, tag="ntp")
            for c in range(NCLS):
                ohT = ohp.tile([128, 2 * L], bf16, tag="ohT")
                nc.vector.tensor_scalar(
                    ohT[:], xmT[:], float(c + 1), None, op0=OP.is_equal
                )
                nc.tensor.matmul(
                    ntp[:], oc_sb[:, 63 - c : 127 - c], ohT[:],
                    start=(c == 0), stop=(c == NCLS - 1),
                )
            nT = work.tile([KPAD, 2 * L], f32, tag="nT")
            nc.scalar.copy(nT[:], ntp[:])

            # y_m[l=128m+p, :] = sum_c (nT[c,h0,l] + nT[c,h1,l]) * G[c, :]
            t1b = work.tile([128, 2 * NF], bf16, tag="t1b")
            for m in range(2):
                y = psum.tile([128, NF + 1], f32, tag=f"y{m}")
                nc.tensor.matmul(
                    y[:], nT[:, 128 * m : 128 * m + 128], g_sb[:],
                    start=True, stop=False,
                )
                nc.tensor.matmul(
                    y[:], nT[:, 2 * L // 2 + 128 * m : 2 * L // 2 + 128 * m + 128],
                    g_sb[:], start=False, stop=True,
                )
                r = work.tile([128, 1], f32, tag=f"r{m}")
                nc.vector.reciprocal(r[:], y[:, NF : NF + 1])
                nc.scalar.activation(
                    t1b[:, m * NF : (m + 1) * NF], y[:, 0:NF], AF.Copy, scale=r[:]
                )
            t2 = work.tile([128, 2 * NF], bf16, tag="t2")
            nc.vector.tensor_tensor(t2[:], t1b[:], txt[:], op=OP.add)
            o = work.tile([128, 2 * NF], f32, tag="o")
            nc.scalar.activation(o[:], t2[:], AF.Relu)
            nc.sync.dma_start(out[:], o[:])

        if reps == 1:
            body()
        elif reps % unroll == 0 and unroll > 1:
            with tc.For_i(0, reps // unroll, 1, staggered_reset=STAGGERED):
                for _ in range(unroll):
                    body()
        else:
            with tc.For_i(0, reps, 1, staggered_reset=STAGGERED):
                body()

    return nc


W174 = 2 * W3                    # packed words per partition, both row-tiles


def _build_nc_v3(reps=1):
    """v3: like v2 but the histogram is ONE custom DVE SWAR-scan instruction
    (plus ~10 small unpack ops) instead of 100 is_equal+accum ops.

    Layout: xm [128, 522] bf16 holds (lab+1)*mask per row-tile in 261-col
    blocks (cols 256..260 of each block are 127-pads).  Packed to bytes
    0..2 of 174 int32 words (byte 3 = 0x7f spacer, set once).  The scan
    streams 51 pages x 174 words; page-end differences give per-(tile,
    class) counts packed at bits 7/15/23."""
    bf16 = mybir.dt.bfloat16
    AF = mybir.ActivationFunctionType
    unroll = int(os.environ.get("UNROLL", "8"))
    hist_op = _register_hist3()
    nc = bass.Bass()
    lme = nc.dram_tensor("lm", [128, 4 * L], bf16, kind="ExternalInput")
    txte = nc.dram_tensor("txt", [128, 2 * NF], bf16, kind="ExternalInput")
    gext = nc.dram_tensor("gext", [KPAD, NF + 1], f32, kind="ExternalInput")
    ident = nc.dram_tensor("ident", [128, 128], f32, kind="ExternalInput")
    quads = nc.dram_tensor("quads", [128, NPAGE], i32, kind="ExternalInput")
    consts = nc.dram_tensor("consts", [128, 8], i32, kind="ExternalInput")
    out = nc.dram_tensor("out", [128, 2 * NF], f32, kind="ExternalOutput")

    with ExitStack() as ctx:
        tc = ctx.enter_context(tile.TileContext(nc))
        const = ctx.enter_context(tc.tile_pool(name="const", bufs=1))
        work = ctx.enter_context(tc.tile_pool(name="work", bufs=3))
        scanp = ctx.enter_context(tc.tile_pool(name="scan", bufs=1))
        psum = ctx.enter_context(tc.tile_pool(name="psum", bufs=2, space="PSUM"))

        def stage_const(shape, dt, src):
            t_dma = const.tile(shape, dt)
            nc.sync.dma_start(t_dma[:], src[:])
            t_sb = const.tile(shape, dt)
            nc.vector.tensor_copy(t_sb[:], t_dma[:])
            return t_sb

        g_sb = stage_const([KPAD, NF + 1], f32, gext)
        id_sb = stage_const([128, 128], f32, ident)
        q_sb = stage_const([128, NPAGE], i32, quads)
        c_sb = stage_const([128, 8], i32, consts)
        c_k80 = c_sb[:, 0:1]
        c_sh7 = c_sb[:, 2:3]
        c_sh15 = c_sb[:, 3:4]
        c_sh23 = c_sb[:, 4:5]
        c_m7f = c_sb[:, 5:6]

        # one-time-initialized scan-pipeline tiles (pads/spacers persist)
        xm = scanp.tile([128, 2 * PK3], bf16, tag="xm")
        nc.vector.memset(xm[:], 127.0)
        xm8 = scanp.tile([128, 4 * W174], i8, tag="xm8")
        nc.vector.memset(xm8[:], 127)
        words = xm8[:].bitcast(i32)              # [128, 174]
        xm8_3 = xm8[:].rearrange("p (w b) -> p w b", b=4)[:, :, 0:3]
        p3 = scanp.tile([128, NPAGE * W174], i32, tag="p3")
        p3v = p3[:].rearrange("p (s n) -> p s n", n=W174)

        def body():
            lm = work.tile([128, 4 * L], bf16, tag="lm")
            nc.sync.dma_start(lm[:], lme[:])
            txt = work.tile([128, 2 * NF], bf16, tag="txt")
            nc.sync.dma_start(txt[:], txte[:])

            # xm block t (cols 261t..261t+255) = (lab+1)*mask for row-tile t
            for t in range(2):
                nc.vector.scalar_tensor_tensor(
                    xm[:, PK3 * t : PK3 * t + L],
                    lm[:, t * L : (t + 1) * L], 1.0,
                    lm[:, 2 * L + t * L : 2 * L + (t + 1) * L],
                    op0=OP.add, op1=OP.mult,
                )
            # pack to bytes 0..2 of each word (byte 3 stays 0x7f)
            nc.vector.tensor_copy(xm8_3, xm[:])

            # one scan: cumulative SWAR match counts over 51 pages x 174 words
            nc.vector._custom_dve(
                hist_op,
                out=p3v,
                in0=words[:, None, :].broadcast_to((128, NPAGE, W174)),
                in1=q_sb[:, :, None].broadcast_to((128, NPAGE, W174)),
                s0=c_k80,
            )

            # per-(tile, class) packed counts via page-boundary diffs
            npk = work.tile([128, 128], i32, tag="npk")
            nc.vector.tensor_tensor(
                npk[:, 0:NCLS, None],
                p3v[:, 1:NPAGE, W3 - 1 : W3],
                p3v[:, 0 : NPAGE - 1, W174 - 1 : W174],
                op=OP.subtract,
            )
            nc.vector.tensor_tensor(
                npk[:, 64 : 64 + NCLS, None],
                p3v[:, 1:NPAGE, W174 - 1 : W174],
                p3v[:, 1:NPAGE, W3 - 1 : W3],
                op=OP.subtract,
            )
            # unpack the three 7-bit lane-count fields and sum them
            a0 = work.tile([128, 128], i32, tag="a0")
            nc.vector.tensor_scalar(
                a0[:], npk[:], c_sh7, c_m7f,
                op0=OP.logical_shift_right, op1=OP.bitwise_and,
            )
            a1 = work.tile([128, 128], i32, tag="a1")
            nc.vector.tensor_scalar(
                a1[:], npk[:], c_sh15, c_m7f,
                op0=OP.logical_shift_right, op1=OP.bitwise_and,
            )
            a2 = work.tile([128, 128], i32, tag="a2")
            nc.vector.tensor_scalar(
                a2[:], npk[:], c_sh23, c_m7f,
                op0=OP.logical_shift_right, op1=OP.bitwise_and,
            )
            s01 = work.tile([128, 128], i32, tag="s01")
            nc.vector.tensor_tensor(s01[:], a0[:], a1[:], op=OP.add)
            n_i = work.tile([128, 128], i32, tag="n_i")
            nc.vector.tensor_tensor(n_i[:], s01[:], a2[:], op=OP.add)
            n = work.tile([128, 128], f32, tag="n")
            nc.vector.tensor_copy(n[:], n_i[:])

            # transpose, matmul, normalize, add text, relu (same as v2)
            ntp = psum.tile([KPAD, 256], f32, tag="ntp")
            for t in range(2):
                nc.tensor.transpose(
                    ntp[:, 128 * t : 128 * t + 128],
                    n[:, 64 * t : 64 * t + KPAD], id_sb[:],
                )
            nT = work.tile([KPAD, 256], f32, tag="nT")
            nc.scalar.copy(nT[:], ntp[:])

            t1b = work.tile([128, 2 * NF], bf16, tag="t1b")
            for t in range(2):
                y = psum.tile([128, NF + 1], f32, tag=f"y{t}")
                nc.tensor.matmul(
                    y[:], nT[:, 128 * t : 128 * t + 128], g_sb[:],
                    start=True, stop=True,
                )
                r = work.tile([128, 1], f32, tag=f"r{t}")
                nc.vector.reciprocal(r[:], y[:, NF : NF + 1])
                nc.scalar.activation(
                    t1b[:, t * NF : (t + 1) * NF], y[:, 0:NF], AF.Copy, scale=r[:]
                )
            t2 = work.tile([128, 2 * NF], bf16, tag="t2")
            nc.vector.tensor_tensor(t2[:], t1b[:], txt[:], op=OP.add)
            o = work.tile([128, 2 * NF], f32, tag="o")
            nc.scalar.activation(o[:], t2[:], AF.Relu)
            nc.sync.dma_start(out[:], o[:])

        if reps == 1:
            body()
        elif reps % unroll == 0 and unroll > 1:
            with tc.For_i(0, reps // unroll, 1, staggered_reset=STAGGERED):
                for _ in range(unroll):
                    body()
        else:
            with tc.For_i(0, reps, 1, staggered_reset=STAGGERED):
                body()

    return nc


def _build_nc(reps=1):
    nc = bass.Bass()
    text = nc.dram_tensor("text", [L, NF], f32, kind="ExternalInput")
    labels = nc.dram_tensor("labels", [L, L], i32, kind="ExternalInput")
    mask = nc.dram_tensor("mask", [L, L], i32, kind="ExternalInput")
    gext = nc.dram_tensor("gext", [KPAD, NF + 1], f32, kind="ExternalInput")
    ident = nc.dram_tensor("ident", [128, 128], f32, kind="ExternalInput")
    if HIST_MODE == "swar3":
        quads = nc.dram_tensor("quads", [128, NPAGE], i32, kind="ExternalInput")
        consts = nc.dram_tensor("consts", [128, 8], i32, kind="ExternalInput")
        hist_op = _register_hist3()
    out = nc.dram_tensor("out", [L, NF], f32, kind="ExternalOutput")

    with ExitStack() as ctx:
        tc = ctx.enter_context(tile.TileContext(nc))
        const = ctx.enter_context(tc.tile_pool(name="const", bufs=1))
        work = ctx.enter_context(tc.tile_pool(name="work", bufs=3))
        psum = ctx.enter_context(tc.tile_pool(name="psum", bufs=2, space="PSUM"))

        # Stage constants through DVE so PE matmuls wait on one engine sem
        # (PE is HW-decoded with few sync-wait slots).
        g_dma = const.tile([KPAD, NF + 1], f32)
        nc.sync.dma_start(g_dma[:], gext[:])
        g_sb = const.tile([KPAD, NF + 1], f32)
        nc.vector.tensor_copy(g_sb[:], g_dma[:])
        if HIST_MODE != "fused":
            id_dma = const.tile([128, 128], f32)
            nc.sync.dma_start(id_dma[:], ident[:])
            id_sb = const.tile([128, 128], f32)
            nc.vector.tensor_copy(id_sb[:], id_dma[:])
        if HIST_MODE == "swar3":
            q_dma = const.tile([128, NPAGE], i32)
            nc.sync.dma_start(q_dma[:], quads[:])
            q_sb = const.tile([128, NPAGE], i32)
            nc.vector.tensor_copy(q_sb[:], q_dma[:])
            c_dma = const.tile([128, 8], i32)
            nc.sync.dma_start(c_dma[:], consts[:])
            c_sb = const.tile([128, 8], i32)
            nc.vector.tensor_copy(c_sb[:], c_dma[:])
            c_k80 = c_sb[:, 0:1]
            c_127 = c_sb[:, 1:2]
            c_sh7 = c_sb[:, 2:3]
            c_sh15 = c_sb[:, 3:4]
            c_sh23 = c_sb[:, 4:5]
            c_m7f = c_sb[:, 5:6]

        if HIST_MODE == "fused":
            bf16 = mybir.dt.bfloat16
            NT = L // 128
            lab_r = labels[:].rearrange("(t p) j -> p t j", p=128)
            msk_r = mask[:].rearrange("(t p) j -> p t j", p=128)
            txt_r = text[:].rearrange("(t p) j -> p t j", p=128)
            out_r = out[:].rearrange("(t p) j -> p t j", p=128)

            def fused_body():
                if PROBE == "empty":
                    z = work.tile([128, 8], f32, tag="z")
                    nc.vector.memset(z[:], 0.0)
                    return
                lab = work.tile([128, NT * L], i32, tag="lab")
                msk = work.tile([128, NT * L], i32, tag="msk")
                txt = work.tile([128, NT * NF], f32, tag="txt")
                if PROBE != "nodma":
                    nc.sync.dma_start(lab[:], lab_r)
                    nc.sync.dma_start(msk[:], msk_r)
                    nc.sync.dma_start(txt[:], txt_r)
                else:
                    nc.vector.memset(lab[:, 0:8], 0)
                    nc.vector.memset(msk[:, 0:8], 0)
                    nc.vector.memset(txt[:, 0:8], 0.0)
                if PROBE == "dma":
                    o_dma = work.tile([128, NT * NF], f32, tag="o_both")
                    nc.vector.memset(o_dma[:, 0:8], 0.0)
                    nc.sync.dma_start(out_r, o_dma[:])
                    return

                # xm = (x+1)*m - 1 in bf16 over both row-tiles at once
                labf = work.tile([128, NT * L], bf16, tag="labf")
                nc.vector.tensor_copy(labf[:], lab[:])
                mskf = work.tile([128, NT * L], bf16, tag="mskf")
                nc.vector.tensor_copy(mskf[:], msk[:])
                t0 = work.tile([128, NT * L], bf16, tag="t0")
                nc.vector.scalar_tensor_tensor(
                    t0[:], labf[:], 1.0, mskf[:], op0=OP.add, op1=OP.mult
                )
                xm = work.tile([128, NT * L], bf16, tag="xm")
                nc.vector.tensor_scalar(xm[:], t0[:], 1.0, None, op0=OP.subtract)

                nb = work.tile([128, NT * KPAD], f32, tag="nb")
                nc.vector.memset(nb[:], 0.0)
                oh = work.tile([128, L], bf16, tag="oh")
                ncls_probe = int(os.environ.get("HIST_CLS", str(NCLS)))
                for t in range(NT):
                    for c in range(ncls_probe):
                        nc.vector.tensor_scalar(
                            oh[:], xm[:, t * L : (t + 1) * L], float(c), 0.0,
                            op0=OP.is_equal, op1=OP.add,
                            accum_out=nb[:, t * KPAD + c : t * KPAD + c + 1],
                        )
                if PROBE == "hist":
                    nc.sync.dma_start(out[0:128, 0:NT * KPAD], nb[:])
                    return
                # per-tile transposed histograms on partitions 0..63
                nTs = []
                for t in range(NT):
                    nT = work.tile([KPAD, 128], f32, tag=f"nT{t}")
                    for bi in range(4):
                        for bj in range(KPAD // 32):
                            nc.vector.transpose(
                                nT[bj * 32 : (bj + 1) * 32, bi * 32 : (bi + 1) * 32],
                                nb[bi * 32 : (bi + 1) * 32,
                                   t * KPAD + bj * 32 : t * KPAD + (bj + 1) * 32],
                            )
                    nTs.append(nT)
                o_both = work.tile([128, NT * NF], f32, tag="o_both")
                for t in range(NT):
                    y = psum.tile([128, NF + 1], f32, tag="y")
                    nc.tensor.matmul(y[:], nTs[t][:], g_sb[:], start=True, stop=True)
                    r = work.tile([128, 1], f32, tag="r")
                    nc.vector.reciprocal(r[:], y[:, NF : NF + 1])
                    t1 = work.tile([128, NF], f32, tag="t1")
                    nc.vector.tensor_scalar(
                        t1[:], y[:, 0:NF], r[:], None, op0=OP.mult
                    )
                    t2 = work.tile([128, NF], f32, tag="t2")
                    nc.vector.tensor_tensor(
                        t2[:], t1[:], txt[:, t * NF : (t + 1) * NF], op=OP.add
                    )
                    nc.vector.tensor_scalar(
                        o_both[:, t * NF : (t + 1) * NF], t2[:], 0.0, None,
                        op0=OP.max,
                    )
                nc.sync.dma_start(out_r, o_both[:])

            if reps == 1:
                fused_body()
            else:
                # Hardware loop: reps execute on-device, keeping the NEFF
                # (and per-call host/axon dispatch) independent of rep count.
                with tc.For_i(0, reps, 1, staggered_reset=STAGGERED):
                    fused_body()
            reps = 0  # skip the per-tile path below

        for rep in range(reps):
          for t in range(L // 128):
            sl = slice(t * 128, (t + 1) * 128)
            lab = work.tile([128, L], i32, tag="lab")
            nc.sync.dma_start(lab[:], labels[sl, :])
            msk = work.tile([128, L], i32, tag="msk")
            nc.sync.dma_start(msk[:], mask[sl, :])
            txt = work.tile([128, NF], f32, tag="txt")
            nc.sync.dma_start(txt[:], text[sl, :])

            if HIST_MODE == "tspacc":
                bf16 = mybir.dt.bfloat16
                # masked labels in bf16: xm = (x+1)*m - 1 (masked -> -1);
                # values <= 50 are exact in bf16.
                labf = work.tile([128, L], bf16, tag="labf")
                nc.vector.tensor_copy(labf[:], lab[:])
                mskf = work.tile([128, L], bf16, tag="mskf")
                nc.vector.tensor_copy(mskf[:], msk[:])
                t0 = work.tile([128, L], bf16, tag="t0")
                nc.vector.scalar_tensor_tensor(
                    t0[:], labf[:], 1.0, mskf[:], op0=OP.add, op1=OP.mult
                )
                xm = work.tile([128, L], bf16, tag="xm")
                nc.vector.tensor_scalar(xm[:], t0[:], 1.0, None, op0=OP.subtract)

                n = work.tile([128, KPAD], f32, tag="n")
                nc.vector.memset(n[:], 0.0)
                oh = work.tile([128, L], bf16, tag="oh")
                ngp = int(os.environ.get("HIST_GP", "0"))
                if ngp:
                    oh_g = work.tile([128, L], bf16, tag="oh_g")
                    n_g = work.tile([128, max(ngp, 1)], f32, tag="n_g")
                    for c in range(ngp):
                        nc.gpsimd.tensor_scalar(
                            oh_g[:], xm[:], float(c), 0.0,
                            op0=OP.is_equal, op1=OP.add,
                            accum_out=n_g[:, c : c + 1],
                        )
                for c in range(ngp, NCLS):
                    nc.vector.tensor_scalar(
                        oh[:], xm[:], float(c), 0.0,
                        op0=OP.is_equal, op1=OP.add,
                        accum_out=n[:, c : c + 1],
                    )
                if ngp:
                    nc.vector.tensor_copy(n[:, 0:ngp], n_g[:])
            elif HIST_MODE == "stock":
                # masked labels in fp32: xm = (x+1)*m - 1  (masked -> -1)
                labf = work.tile([128, L], f32, tag="labf")
                nc.vector.tensor_copy(labf[:], lab[:])
                mskf = work.tile([128, L], f32, tag="mskf")
                nc.vector.tensor_copy(mskf[:], msk[:])
                t0 = work.tile([128, L], f32, tag="t0")
                nc.vector.scalar_tensor_tensor(
                    t0[:], labf[:], 1.0, mskf[:], op0=OP.add, op1=OP.mult
                )
                xm = work.tile([128, L], f32, tag="xm")
                nc.vector.tensor_scalar(xm[:], t0[:], 1.0, None, op0=OP.subtract)

                # histogram n[l, c]
                n = work.tile([128, KPAD], f32, tag="n")
                nc.vector.memset(n[:], 0.0)
                oh = work.tile([128, L], f32, tag="oh")
                for c in range(NCLS):
                    nc.vector.tensor_scalar(
                        oh[:], xm[:], float(c), None, op0=OP.is_equal
                    )
                    nc.vector.tensor_reduce(
                        n[:, c : c + 1], oh[:], axis=AX.X, op=OP.add
                    )
            else:
                # xm_i32 = (lab - 127) * msk + 127  (masked -> 127 = 0x7f dummy)
                t0 = work.tile([128, L], i32, tag="t0")
                nc.vector.scalar_tensor_tensor(
                    t0[:], lab[:], 127.0, msk[:], op0=OP.subtract, op1=OP.mult
                )
                t1 = work.tile([128, PK3 + 3], i32, tag="t1")
                nc.vector.memset(t1[:], 127)
                nc.vector.tensor_scalar(t1[:, 0:L], t0[:], 127.0, None, op0=OP.add)

                # pack 3 labels per int32 word, byte 3 = 0x7f dummy
                xm8 = work.tile([128, 4 * W3], i8, tag="xm8")
                nc.vector.memset(xm8[:], 127)
                xm8_3 = xm8[:].rearrange("p (w b) -> p w b", b=4)[:, :, 0:3]
                nc.vector.tensor_copy(xm8_3, t1[:, 0:PK3])
                words = xm8[:].bitcast(i32)          # [128, W3]

                prefix = work.tile([128, NPAGE * W3], i32, tag="prefix")
                p3 = prefix[:].rearrange("p (s n) -> p s n", n=W3)
                nc.vector._custom_dve(
                    hist_op,
                    out=p3,
                    in0=words[:, None, :].broadcast_to((128, NPAGE, W3)),
                    in1=q_sb[:, :, None].broadcast_to((128, NPAGE, W3)),
                    s0=c_k80,
                )

                # per-class packed lane counts = page-end diffs
                npack = work.tile([128, NCLS], i32, tag="npack")
                nc.vector.tensor_tensor(
                    npack[:, :, None],
                    p3[:, 1:NPAGE, W3 - 1 : W3],
                    p3[:, 0 : NPAGE - 1, W3 - 1 : W3],
                    op=OP.subtract,
                )
                # unpack marks at bits 7/15/23 (each lane count < 128)
                a0 = work.tile([128, NCLS], i32, tag="a0")
                nc.vector.tensor_scalar(
                    a0[:], npack[:], c_sh7, c_m7f,
                    op0=OP.logical_shift_right, op1=OP.bitwise_and,
                )
                a1 = work.tile([128, NCLS], i32, tag="a1")
                nc.vector.tensor_scalar(
                    a1[:], npack[:], c_sh15, c_m7f,
                    op0=OP.logical_shift_right, op1=OP.bitwise_and,
                )
                a2 = work.tile([128, NCLS], i32, tag="a2")
                nc.vector.tensor_scalar(
                    a2[:], npack[:], c_sh23, c_m7f,
                    op0=OP.logical_shift_right, op1=OP.bitwise_and,
                )
                s01 = work.tile([128, NCLS], i32, tag="s01")
                nc.vector.tensor_tensor(s01[:], a0[:], a1[:], op=OP.add)
                n_i = work.tile([128, NCLS], i32, tag="n_i")
                nc.vector.tensor_tensor(n_i[:], s01[:], a2[:], op=OP.add)
                n = work.tile([128, KPAD], f32, tag="n")
                nc.vector.memset(n[:], 0.0)
                nc.vector.tensor_copy(n[:, 0:NCLS], n_i[:])

            # n^T: DVE 32x32 block transpose (avoids the PE is_transpose
            # XBAR path) unless TRANSPOSE_MODE=pe
            nT = work.tile([KPAD, 128], f32, tag="nT")
            if os.environ.get("TRANSPOSE_MODE", "dve") == "dve":
                for bi in range(4):
                    for bj in range(KPAD // 32):
                        nc.vector.transpose(
                            nT[bj * 32 : (bj + 1) * 32, bi * 32 : (bi + 1) * 32],
                            n[bi * 32 : (bi + 1) * 32, bj * 32 : (bj + 1) * 32],
                        )
            else:
                ntp = psum.tile([KPAD, 128], f32, tag="ntp")
                nc.tensor.transpose(ntp[:], n[:], id_sb[:])
                nc.vector.tensor_copy(nT[:], ntp[:])

            # Y[l, :] = n[l, :] @ G_ext  -> [128, NF+1]; col NF is rowsum
            y = psum.tile([128, NF + 1], f32, tag="y")
            nc.tensor.matmul(y[:], nT[:], g_sb[:], start=True, stop=True)

            r = work.tile([128, 1], f32, tag="r")
            nc.vector.reciprocal(r[:], y[:, NF : NF + 1])
            t1 = work.tile([128, NF], f32, tag="t1")
            nc.vector.tensor_scalar(t1[:], y[:, 0:NF], r[:], None, op0=OP.mult)
            t2 = work.tile([128, NF], f32, tag="t2")
            nc.vector.tensor_tensor(t2[:], t1[:], txt[:], op=OP.add)
            o = work.tile([128, NF], f32, tag="o")
            nc.vector.tensor_scalar(o[:], t2[:], 0.0, None, op0=OP.max)
            nc.sync.dma_start(out[sl, :], o[:])

    return nc


_ASYNC_INST = ("InstDMACopy", "InstTrigger", "InstCollective", "InstISA")


def _strip_same_engine_waits(nc):
    """Remove embedded semaphore waits that only guard program order on the
    SAME engine queue.  Tile emits a wait (own-engine sem >= k) plus inc on
    every instruction; engines execute their queue serially, so the data
    hazard is already resolved, but each wait stalls until the previous
    op's sem-inc lands in the sync fabric (~0.5us/op).  A wait is stripped
    iff every updater of its semaphore is a synchronous compute instruction
    on the waiting instruction's own engine (async updaters - DMA,
    collectives, raw ISA - keep their waits)."""
    sem_updaters = {}
    for fn in nc.m.functions:
        for blk in fn.blocks:
            for inst in blk.instructions:
                si = inst.sync_info
                if si is None:
                    continue
                for u in si.on_update:
                    ent = sem_updaters.setdefault(u.id, {"engines": set(), "ok": True})
                    ent["engines"].add(inst.engine)
                    tn = type(inst).__name__
                    if any(tn.startswith(p) for p in _ASYNC_INST):
                        ent["ok"] = False
                    if getattr(u, "update_mode", "sem-inc") not in (
                        "sem-inc", "sem-add-imm", "sem-sub-imm"
                    ):
                        ent["ok"] = False
    n_strip = 0
    for fn in nc.m.functions:
        for blk in fn.blocks:
            for inst in blk.instructions:
                si = inst.sync_info
                if si is None or not si.on_wait:
                    continue
                kept = []
                for w in si.on_wait:
                    ent = sem_updaters.get(w.id)
                    if (
                        ent is not None
                        and ent["ok"]
                        and ent["engines"] == {inst.engine}
                        and getattr(w, "wait_mode", None) == "sem-ge-imm"
                        and getattr(w, "wait_reg", None) is None
                    ):
                        n_strip += 1
                        continue
                    kept.append(w)
                if len(kept) != len(si.on_wait):
                    inst.sync_info = mybir.SyncInfo(
                        on_wait=kept, on_update=si.on_update
                    )
    return nc


def _thin_engine_updates(nc):
    """Keep only milestone semaphore increments on per-engine queue sems.
    A sem-inc is needed only where some waiter's wait_value is first
    reached, and at each block's last inc (so block-end counts - and the
    back-edge sem-sub reset arithmetic - stay exactly as Tile computed
    them).  Intermediate incs are dropped, their amounts folded into the
    next kept inc, preserving every wait's trigger point."""
    sem_info = {}
    for fn in nc.m.functions:
        for blk in fn.blocks:
            for inst in blk.instructions:
                si = inst.sync_info
                if si is None:
                    continue
                for u in si.on_update:
                    ent = sem_info.setdefault(u.id, {"engines": set(), "ok": True})
                    ent["engines"].add(inst.engine)
                    tn = type(inst).__name__
                    if any(tn.startswith(p) for p in _ASYNC_INST):
                        ent["ok"] = False
                    if getattr(u, "update_mode", None) not in (
                        "sem-inc", "sem-sub-imm", "sem-add-imm"
                    ):
                        ent["ok"] = False
    thin = {
        sid for sid, ent in sem_info.items()
        if ent["ok"] and len(ent["engines"]) == 1
    }
    waited = {}
    for fn in nc.m.functions:
        for blk in fn.blocks:
            for inst in blk.instructions:
                si = inst.sync_info
                if si is None:
                    continue
                for w in si.on_wait:
                    if w.id in thin and getattr(w, "wait_value", None) is not None:
                        waited.setdefault(w.id, set()).add(w.wait_value)
    for fn in nc.m.functions:
        run = {}                       # sem -> absolute count (carried)
        for blk in fn.blocks:
            insts = blk.instructions
            plans = {}                 # inst_idx -> [(upd_idx, new_val)]
            pending = {}               # sem -> folded dropped incs
            last_inc_site = {}         # sem -> (inst_idx, upd_idx)
            for ii, inst in enumerate(insts):
                si = inst.sync_info
                if si is None:
                    continue
                for ui, u in enumerate(si.on_update):
                    if u.id not in thin:
                        continue
                    mode = getattr(u, "update_mode", None)
                    if mode == "sem-sub-imm":
                        run[u.id] = run.get(u.id, 0) - u.update_value
                        continue
                    if mode not in ("sem-inc", "sem-add-imm"):
                        continue
                    prev = run.get(u.id, 0)
                    c = prev + u.update_value
                    run[u.id] = c
                    pend = pending.get(u.id, 0) + u.update_value
                    vs = waited.get(u.id, ())
                    if any(prev < v <= c for v in vs):
                        plans.setdefault(ii, []).append((ui, pend))
                        pending[u.id] = 0
                    else:
                        plans.setdefault(ii, []).append((ui, 0))
                        pending[u.id] = pend
                    last_inc_site[u.id] = (ii, ui)
            # restore block-end totals: fold leftovers into the last inc
            for sid, pend in pending.items():
                if pend and sid in last_inc_site:
                    ii, ui = last_inc_site[sid]
                    lst = plans.setdefault(ii, [])
                    for k, (uidx, val) in enumerate(lst):
                        if uidx == ui:
                            lst[k] = (uidx, val + pend)
                            break
                    else:
                        lst.append((ui, pend))
            for ii, edits in plans.items():
                inst = insts[ii]
                si = inst.sync_info
                new_updates = list(si.on_update)
                drop = []
                for ui, val in edits:
                    u = new_updates[ui]
                    if val == 0:
                        drop.append(ui)
                    elif val != u.update_value:
                        new_updates[ui] = mybir.SyncUpdate(
                            sync_type=u.sync_type, id=u.id,
                            ant_name=u.ant_name,
                            update_mode=u.update_mode, update_value=val,
                        )
                for ui in sorted(drop, reverse=True):
                    del new_updates[ui]
                inst.sync_info = mybir.SyncInfo(
                    on_wait=si.on_wait, on_update=new_updates
                )
    return nc


def _legalize_waits(nc):
    """This walrus build accepts at most one embedded SyncWait per engine
    instruction; hoist extras into standalone sequencer EventSemaphore
    instructions (what raw-bass wait_ge emits)."""
    k = 0
    for fn in nc.m.functions:
        for blk in fn.blocks:
            new_insts = []
            for inst in blk.instructions:
                si = inst.sync_info
                if si is not None and len(si.on_wait) > 1:
                    for w in si.on_wait[:-1]:
                        k += 1
                        ev = mybir.InstEventSemaphore(
                            name=f"EVW-{k}",
                            engine=inst.engine,
                            ins=[],
                            outs=[],
                            sync_info=mybir.SyncInfo(on_wait=[w], on_update=[]),
                            bass_nofuse=True,
                        )
                        new_insts.append(ev)
                    inst.sync_info = mybir.SyncInfo(
                        on_wait=[si.on_wait[-1]], on_update=si.on_update
                    )
                new_insts.append(inst)
            del blk.instructions[:]
            blk.instructions.extend(new_insts)
    return nc


_NC_CACHE = {}


def _get_nc(reps=1):
    key = (HIST_MODE, reps)
    if key not in _NC_CACHE:
        build = {"v2": _build_nc_v2, "v3": _build_nc_v3, "v4": _build_nc_v4}.get(
            HIST_MODE, _build_nc
        )
        nc = build(reps)
        if os.environ.get("KEEP_WAITS", "0") != "1":
            nc = _strip_same_engine_waits(nc)
        if os.environ.get("THIN", "0") == "1":
            nc = _thin_engine_updates(nc)
        _NC_CACHE[key] = _legalize_waits(nc)
    return _NC_CACHE[key]


def _host_consts(dep_emb, W_attn, b_attn, W_fc, b_fc):
    dep_emb = np.asarray(dep_emb, np.float64)
    W_attn = np.asarray(W_attn, np.float64)
    W_fc = np.asarray(W_fc, np.float64)
    b_fc = np.asarray(b_fc, np.float64)
    wa_dep = W_attn[NF:].sum(axis=1)            # [64]
    s_tab = dep_emb @ wa_dep                    # [50]
    E = np.exp(s_tab - s_tab.max())             # [50]
    M = dep_emb @ W_fc                          # [50, 256]
    G2 = E[:, None] * (M + b_fc[None, :])       # [50, 256]
    gext = np.zeros([KPAD, NF + 1], np.float32)
    gext[:NCLS, :NF] = G2.astype(np.float32)
    gext[:NCLS, NF] = E.astype(np.float32)
    return gext


def run(inputs, trace=False, reps=1):
    text = np.ascontiguousarray(np.asarray(inputs["text"], np.float32))
    dep_mat = np.ascontiguousarray(np.asarray(inputs["dep_mat"], np.int32))
    dep_labels = np.ascontiguousarray(np.asarray(inputs["dep_labels"], np.int32))
    gext = _host_consts(
        inputs["dep_emb"], inputs["W_attn"], inputs["b_attn"],
        inputs["W_fc"], inputs["b_fc"],
    )
    ident = np.eye(128, dtype=np.float32)

    if HIST_MODE in ("v2", "v3", "v4"):
        import ml_dtypes

        bf = ml_dtypes.bfloat16
        nc = _get_nc(reps)
        extra = {}
        if HIST_MODE == "v3":
            q = np.empty(NPAGE, np.int64)
            q[0] = 0x40404040              # anchor page: matches nothing
            for c in range(NCLS):
                q[c + 1] = (c + 1) * 0x00010101
            extra["quads"] = np.broadcast_to(
                q.astype(np.int32), (128, NPAGE)
            ).copy()
            cvals = np.zeros(8, np.int64)
            cvals[0] = K80
            cvals[1] = 127
            cvals[2] = 7
            cvals[3] = 15
            cvals[4] = 23
            cvals[5] = 0x7F
            extra["consts"] = np.broadcast_to(
                cvals.astype(np.int32), (128, 8)
            ).copy()
        actc_host = np.broadcast_to(
            -(np.arange(KPAD, dtype=np.float32) + 1.0), (128, KPAD)
        ).copy()
        ohc_host = np.zeros((128, 128), ml_dtypes.bfloat16)
        ohc_host[:, 63] = 1.0
        in_maps = []
        for b in range(B):
            if HIST_MODE == "v4":
                # transposed: partition = neighbor j (mod 128), free = (jhalf, l)
                labT = dep_labels[b].T
                mskT = dep_mat[b].T
                lab2 = np.concatenate([labT[0:128, :], labT[128:256, :]], axis=1)
                msk2 = np.concatenate([mskT[0:128, :], mskT[128:256, :]], axis=1)
            else:
                # [128, 2*L]: cols 0:L = rows 0..127, cols L:2L = rows 128..255
                lab2 = np.concatenate(
                    [dep_labels[b, 0:128, :], dep_labels[b, 128:256, :]], axis=1
                )
                msk2 = np.concatenate(
                    [dep_mat[b, 0:128, :], dep_mat[b, 128:256, :]], axis=1
                )
            lm = np.concatenate([lab2, msk2], axis=1).astype(bf)
            txt = np.concatenate(
                [text[b, 0:128, :], text[b, 128:256, :]], axis=1
            ).astype(bf)
            m = {"lm": np.ascontiguousarray(lm),
                 "txt": np.ascontiguousarray(txt),
                 "gext": gext, **extra}
            if HIST_MODE == "v4":
                m["ohc"] = ohc_host
            else:
                m["ident"] = ident
            if HIST_MODE == "v2":
                m["actc"] = actc_host
            in_maps.append(m)
        res = run_bass_kernel_spmd(nc, in_maps, list(range(B)), trace=trace)
        # out is [128, 2*NF] partition-major; reassemble rows 0..255
        out = np.empty((B, L, NF), np.float32)
        for b in range(B):
            ob = res.results[b]["out"]
            out[b, 0:128, :] = ob[:, 0:NF]
            out[b, 128:256, :] = ob[:, NF : 2 * NF]
        return out, res

    nc = _get_nc(reps)
    extra = {}
    if HIST_MODE == "swar3":
        q = np.empty(NPAGE, np.int64)
        q[0] = 0x40404040              # no-match page (labels<=0x31, dummy 0x7f)
        for c in range(NCLS):
            q[c + 1] = c * 0x00010101
        extra["quads"] = np.broadcast_to(
            q.astype(np.int32), (128, NPAGE)
        ).copy()
        cvals = np.zeros(8, np.int64)
        cvals[0] = K80
        cvals[1] = 127
        cvals[2] = 7
        cvals[3] = 15
        cvals[4] = 23
        cvals[5] = 0x7F
        extra["consts"] = np.broadcast_to(
            cvals.astype(np.int32), (128, 8)
        ).copy()
    in_maps = [
        {
            "text": text[b],
            "labels": dep_labels[b],
            "mask": dep_mat[b],
            "gext": gext,
            "ident": ident,
            **extra,
        }
        for b in range(B)
    ]
    res = run_bass_kernel_spmd(nc, in_maps, list(range(B)), trace=trace)
    out = np.stack([res.results[b]["out"] for b in range(B)])
    return out, res


def kernel(**inputs) -> np.ndarray:
    out, _ = run(inputs, trace=False)
    return out

